# revision 1
# baseline (speedup 1.0000x reference)
"""Trainium2 Bass kernel for a dense transformer block (nn_Block_88338887344891).

Distribution over 8 NeuronCores (single SPMD NEFF, 3 collectives):
  - LayerNorm1 token-sharded (512 tokens/core, feature-major) -> AllGather h^T
  - QKV projection head-sharded (2 heads/core), causal attention per (batch, head)
  - attention output AllToAll per head (head-shard -> token-shard), overlapped
  - output projection + residual + LN2 + full MLP token-sharded (no collective)

All matmuls run as float32r (TF32 mantissa, fp32 accumulate) at 1 cycle/row.
Activations are feature-major ([D on partitions, tokens free]) so per-token
LayerNorm/softmax reductions are ones-matmuls on the PE and per-feature params
are natural per-partition scalars. The two heads of a core are stacked on
partitions 0-63 / 64-127; score matmuls for h0/h1 then occupy disjoint PE
row-groups (auto tile_position) and run concurrently, and one Exp covers both.
"""
import numpy as np
from contextlib import ExitStack

try:  # persistent XLA cache so repeat runs skip the NEFF compile
    import jax
    jax.config.update("jax_compilation_cache_dir", "/tmp/jax_neff_cache")
    jax.config.update("jax_persistent_cache_min_compile_time_secs", 1.0)
except Exception:
    pass

import concourse.bass as bass
import concourse.bacc as bacc
import concourse.tile as tile
import concourse.mybir as mybir
from concourse.masks import make_identity
from concourse import bass_utils

AF = mybir.ActivationFunctionType
ALU = mybir.AluOpType
F32 = mybir.dt.float32
F32R = mybir.dt.float32r

NC_N = 8          # cores
B, T, D, H = 2, 2048, 1024, 16
HD = D // H       # 64
DFF = 4 * D       # 4096
EPS = 1e-5
TPC = (B * T) // NC_N    # 512 tokens per core
HPC = H // NC_N          # 2 heads per core
PO = D // 128            # 8 D-tiles
M1 = DFF // 128          # 32 ff1 out tiles
C_GELU = float(np.sqrt(2.0 / np.pi))
RG = [list(range(NC_N))]

# Native ACT gelu table (1 op) for hardware; CoreSim doesn't implement it,
# so sim runs (test.py --sim / analyze.py) flip this to the composed form.
GELU_NATIVE = True

_CACHE = {}


def _build():
    nc = bacc.Bacc("TRN2", target_bir_lowering=False, debug=False,
                   num_devices=NC_N)

    # ---- per-core external inputs ----
    xt_in = nc.dram_tensor("xt", [D, TPC], F32R, kind="ExternalInput")
    ln1w_in = nc.dram_tensor("ln1w", [128, PO], F32, kind="ExternalInput")
    ln1b_in = nc.dram_tensor("ln1b", [128, PO], F32, kind="ExternalInput")
    ln2w_in = nc.dram_tensor("ln2w", [128, PO], F32, kind="ExternalInput")
    ln2b_in = nc.dram_tensor("ln2b", [128, PO], F32, kind="ExternalInput")
    wqk_in = nc.dram_tensor("wqk", [D, 256], F32R, kind="ExternalInput")
    bqk_in = nc.dram_tensor("bqk", [128, 2], F32, kind="ExternalInput")
    wv_in = nc.dram_tensor("wv", [D, 128], F32R, kind="ExternalInput")
    bv_in = nc.dram_tensor("bv", [128, 1], F32, kind="ExternalInput")
    wo_in = nc.dram_tensor("wo", [PO, D, 128], F32R, kind="ExternalInput")
    bo_in = nc.dram_tensor("bo", [128, PO], F32, kind="ExternalInput")
    wf1_in = nc.dram_tensor("wf1", [M1, D, 128], F32R, kind="ExternalInput")
    bf1_in = nc.dram_tensor("bf1", [128, M1], F32, kind="ExternalInput")
    wf2_in = nc.dram_tensor("wf2", [PO, DFF, 128], F32R, kind="ExternalInput")
    bf2_in = nc.dram_tensor("bf2", [128, PO], F32, kind="ExternalInput")
    out_t = nc.dram_tensor("outt", [D, TPC], F32, kind="ExternalOutput")

    with tile.TileContext(nc) as tc, ExitStack() as ctx:
        perm = ctx.enter_context(tc.tile_pool(name="perm", bufs=1))
        big = ctx.enter_context(tc.tile_pool(name="big", bufs=1))
        psum = ctx.enter_context(tc.tile_pool(name="psum", bufs=4, space="PSUM"))
        ps2 = ctx.enter_context(tc.tile_pool(name="ps2", bufs=2, space="PSUM"))
        rows = ctx.enter_context(tc.tile_pool(name="rows", bufs=1))
        sqp = ctx.enter_context(tc.tile_pool(name="sqp", bufs=2))
        dram = ctx.enter_context(tc.tile_pool(name="dram", bufs=1, space="DRAM"))

        # ---- constants ----
        ones_col_f = perm.tile([128, 1], F32)
        nc.vector.memset(ones_col_f[:], 1.0)
        ones_col_r = perm.tile([128, 1], F32R)
        nc.vector.tensor_copy(ones_col_r[:], ones_col_f[:])
        ones_row_f = perm.tile([1, 128], F32)
        nc.vector.memset(ones_row_f[:], 1.0)
        ones_row_r = perm.tile([1, 128], F32R)
        nc.vector.tensor_copy(ones_row_r[:], ones_row_f[:])
        ident = perm.tile([128, 128], F32)
        make_identity(nc, ident[:])

        def load_const(t_in, shape, tag):
            t = perm.tile(shape, F32, tag=tag)
            nc.sync.dma_start(t[:], t_in.ap())
            return t

        ln1w = load_const(ln1w_in, [128, PO], "c_ln1w")
        ln1b = load_const(ln1b_in, [128, PO], "c_ln1b")
        ln2w = load_const(ln2w_in, [128, PO], "c_ln2w")
        ln2b = load_const(ln2b_in, [128, PO], "c_ln2b")
        bqk = load_const(bqk_in, [128, 2], "c_bqk")
        bv = load_const(bv_in, [128, 1], "c_bv")
        bo = load_const(bo_in, [128, PO], "c_bo")
        bf1 = load_const(bf1_in, [128, M1], "c_bf1")
        bf2 = load_const(bf2_in, [128, PO], "c_bf2")

        X1 = big.tile([128, PO, TPC], F32R, tag="x1")
        nc.sync.dma_start(X1[:], xt_in.ap().rearrange("(po p) t -> p po t", p=128))

        def layernorm(X, w_sb, b_sb, Hout):
            """Feature-major LN over partition(D) axis; X, Hout [128, PO, TPC]."""
            ps_s = psum.tile([1, TPC], F32, tag="ps")
            for po in range(PO):
                nc.tensor.matmul(ps_s[:], ones_col_r[:], X[:, po, :],
                                 start=(po == 0), stop=(po == PO - 1))
            ps_q = psum.tile([1, TPC], F32, tag="ps")
            for po in range(PO):
                sq = sqp.tile([128, TPC], F32R, tag="sq")
                nc.vector.tensor_mul(sq[:], X[:, po, :], X[:, po, :])
                nc.tensor.matmul(ps_q[:], ones_col_r[:], sq[:],
                                 start=(po == 0), stop=(po == PO - 1))
            mu = rows.tile([1, TPC], F32R, tag="mu")
            nc.scalar.activation(mu[:], ps_s[:], AF.Copy, scale=1.0 / D)
            ex2 = rows.tile([1, TPC], F32, tag="ex2")
            nc.scalar.activation(ex2[:], ps_q[:], AF.Copy, scale=1.0 / D)
            var = rows.tile([1, TPC], F32, tag="var")
            nc.vector.tensor_mul(var[:], mu[:].bitcast(F32), mu[:].bitcast(F32))
            nc.vector.tensor_sub(var[:], ex2[:], var[:])
            nc.vector.tensor_scalar_add(var[:], var[:], EPS)
            rec = rows.tile([1, TPC], F32, tag="rec")
            nc.vector.reciprocal(rec[:], var[:])
            inv = rows.tile([1, TPC], F32R, tag="inv")
            nc.scalar.activation(inv[:], rec[:], AF.Sqrt)
            ps_mu = psum.tile([128, TPC], F32, tag="ps")
            nc.tensor.matmul(ps_mu[:], ones_row_r[:], mu[:], start=True, stop=True)
            ps_inv = psum.tile([128, TPC], F32, tag="ps")
            nc.tensor.matmul(ps_inv[:], ones_row_r[:], inv[:], start=True, stop=True)
            for po in range(PO):
                t1 = Hout[:, po, :]
                nc.vector.tensor_sub(t1, X[:, po, :], ps_mu[:])
                nc.vector.tensor_mul(t1, t1, ps_inv[:])
                nc.vector.tensor_scalar(
                    out=t1, in0=t1, scalar1=w_sb[:, po:po + 1],
                    scalar2=b_sb[:, po:po + 1], op0=ALU.mult, op1=ALU.add)

        # ---- Phase 1: LN1 + AllGather h^T ----
        agi = dram.tile([D, TPC], F32R)
        agg = dram.tile([NC_N, D, TPC], F32R, addr_space="Shared")
        H1 = big.tile([128, PO, TPC], F32R, tag="h12")
        layernorm(X1, ln1w, ln1b, H1)
        nc.sync.dma_start(agi[:].rearrange("(po p) t -> p po t", p=128), H1[:])
        nc.gpsimd.collective_compute(
            "AllGather", ALU.bypass, replica_groups=RG,
            ins=[agi[:].opt()], outs=[agg[:].opt()])

        # ---- Phase 2: QKV (head-sharded, heads stacked on partitions) ----
        with tc.tile_pool(name="attn", bufs=1) as attn:
            QT = attn.tile([128, NC_N, TPC], F32R)
            KT = attn.tile([128, NC_N, TPC], F32R)
            Vt = attn.tile([128, 32, HPC, 65], F32R)
            nc.vector.tensor_copy(Vt[:, :, :, 64:65],
                                  ones_col_f[:].to_broadcast([128, 32, HPC, 1]))

            with tc.tile_pool(name="wqkv", bufs=1) as wqkv, \
                 tc.tile_pool(name="hcp", bufs=2) as hcp, \
                 tc.tile_pool(name="vtp", bufs=2) as vtp:
                wqk_sb = wqkv.tile([128, PO, 256], F32R)
                nc.sync.dma_start(
                    wqk_sb[:], wqk_in.ap().rearrange("(po p) m -> p po m", p=128))
                wv_sb = wqkv.tile([128, PO, 128], F32R)
                nc.sync.dma_start(
                    wv_sb[:], wv_in.ap().rearrange("(po p) m -> p po m", p=128))
                for c in range(NC_N):
                    Hc = hcp.tile([128, PO, TPC], F32R, tag="hc")
                    nc.sync.dma_start(
                        Hc[:], agg[c].rearrange("(po p) t -> p po t", p=128))
                    for m, DST in ((0, QT), (1, KT)):
                        psqk = psum.tile([128, TPC], F32, tag="ps")
                        for po in range(PO):
                            nc.tensor.matmul(
                                psqk[:], wqk_sb[:, po, 128 * m:128 * m + 128],
                                Hc[:, po, :],
                                start=(po == 0), stop=(po == PO - 1))
                        nc.vector.tensor_scalar_add(
                            DST[:, c, :], psqk[:], bqk[:, m:m + 1])
                    psv = psum.tile([128, TPC], F32, tag="ps")
                    for po in range(PO):
                        nc.tensor.matmul(psv[:], wv_sb[:, po, :], Hc[:, po, :],
                                         start=(po == 0), stop=(po == PO - 1))
                    vt_t = vtp.tile([128, TPC], F32, tag="vtt")
                    nc.vector.tensor_scalar_add(vt_t[:], psv[:], bv[:])
                    for tt in range(4):
                        g = 4 * c + tt
                        pst = psum.tile([128, 128], F32, tag="ps")
                        nc.tensor.transpose(
                            pst[:], vt_t[:, 128 * tt:128 * tt + 128], ident[:])
                        for h in range(HPC):
                            nc.vector.tensor_copy(
                                Vt[:, g, h, 0:64], pst[:, 64 * h:64 * h + 64])

            # ---- Phase 3: causal attention per (head, batch) ----
            # both heads' scores packed in one 2-bank psum + one Exp; the
            # h0/h1 score matmuls hit disjoint PE row groups and overlap.
            a2ai = dram.tile([NC_N, 128, TPC], F32R)
            a2ao = dram.tile([NC_N, 128, TPC], F32R)
            with tc.tile_pool(name="ptp", bufs=1) as ptp, \
                 tc.tile_pool(name="avp", bufs=2) as avp:
                for b in range(B):
                    for j in range(4):
                        n_kt = 4 * j + 4
                        PT = ptp.tile([128, 16, 2 * TPC], F32R, tag="pt")
                        for i in range(n_kt):
                            pss = ps2.tile([128, 2 * TPC], F32, tag="ps2")
                            cb = 4 * b + i // 4
                            off = (i % 4) * 128
                            for h in range(HPC):
                                nc.tensor.matmul(
                                    pss[:, h * TPC:(h + 1) * TPC],
                                    KT[64 * h:64 * h + 64, cb, off:off + 128],
                                    QT[64 * h:64 * h + 64, 4 * b + j, :],
                                    start=True, stop=True)
                            nc.scalar.activation(PT[:, i, :], pss[:],
                                                 AF.Exp, scale=0.125)
                            if i >= 4 * j:
                                nc.gpsimd.affine_select(
                                    out=PT[:, i, :].rearrange(
                                        "p (h q) -> p h q", h=HPC),
                                    in_=PT[:, i, :].rearrange(
                                        "p (h q) -> p h q", h=HPC),
                                    compare_op=ALU.is_ge, fill=0.0,
                                    base=-128 * (i - 4 * j),
                                    pattern=[[0, HPC], [1, TPC]],
                                    channel_multiplier=-1)
                        for h in range(HPC):
                            ps_av = psum.tile([65, TPC], F32, tag="ps")
                            for i in range(n_kt):
                                nc.tensor.matmul(
                                    ps_av[:], Vt[:, 16 * b + i, h, :],
                                    PT[:, i, h * TPC:(h + 1) * TPC],
                                    start=(i == 0), stop=(i == n_kt - 1))
                            avs = avp.tile([65, TPC], F32R, tag="avs")
                            nc.vector.tensor_copy(avs[:], ps_av[:])
                            rec = avp.tile([1, TPC], F32, tag="avrec")
                            nc.vector.reciprocal(rec[:], avs[64:65, :].bitcast(F32))
                            recr = avp.tile([1, TPC], F32R, tag="avrecr")
                            nc.vector.tensor_copy(recr[:], rec[:])
                            ps_bc = psum.tile([64, TPC], F32, tag="ps")
                            nc.tensor.matmul(ps_bc[:], ones_row_r[:, 0:64],
                                             recr[:], start=True, stop=True)
                            avn = avp.tile([64, TPC], F32R, tag="avn")
                            nc.vector.tensor_mul(avn[:], avs[0:64, :], ps_bc[:])
                            nc.sync.dma_start(
                                a2ai[4 * b + j, 64 * h:64 * h + 64, :], avn[:])

            nc.gpsimd.collective_compute(
                "AllToAll", ALU.bypass, replica_groups=RG,
                ins=[a2ai[:].opt()], outs=[a2ao[:].opt()])

        # ---- Phase 4: output projection + residual ----
        x2p = ctx.enter_context(tc.tile_pool(name="x2p", bufs=1))
        X2 = x2p.tile([128, PO, TPC], F32R, tag="x2")
        with tc.tile_pool(name="avtp", bufs=1) as avtp, \
             tc.tile_pool(name="wop", bufs=2) as wop:
            AVt = avtp.tile([128, NC_N, TPC], F32R)
            nc.sync.dma_start(AVt[:], a2ao[:].rearrange("s p t -> p s t"))
            for m in range(PO):
                wom = wop.tile([128, PO, 128], F32R, tag="wom")
                nc.sync.dma_start(
                    wom[:], wo_in.ap()[m].rearrange("(po p) n -> p po n", p=128))
                ps_o = psum.tile([128, TPC], F32, tag="ps")
                for po in range(PO):
                    nc.tensor.matmul(ps_o[:], wom[:, po, :], AVt[:, po, :],
                                     start=(po == 0), stop=(po == PO - 1))
                nc.vector.tensor_scalar_add(X2[:, m, :], ps_o[:], bo[:, m:m + 1])
                nc.vector.tensor_add(X2[:, m, :], X2[:, m, :], X1[:, m, :])

        # ---- Phase 5: LN2 + MLP ----
        H2 = big.tile([128, PO, TPC], F32R, tag="h12")
        layernorm(X2, ln2w, ln2b, H2)

        with tc.tile_pool(name="ap", bufs=1) as ap_pool, \
             tc.tile_pool(name="w1p", bufs=3) as w1p, \
             tc.tile_pool(name="w2p", bufs=2) as w2p, \
             tc.tile_pool(name="gp", bufs=2) as gp, \
             tc.tile_pool(name="outp", bufs=2) as outp:
            A = ap_pool.tile([128, M1, TPC], F32R)
            for m in range(M1):
                w1m = w1p.tile([128, PO, 128], F32R, tag="w1")
                nc.sync.dma_start(
                    w1m[:], wf1_in.ap()[m].rearrange("(po p) n -> p po n", p=128))
                ps1 = psum.tile([128, TPC], F32, tag="ps")
                for po in range(PO):
                    nc.tensor.matmul(ps1[:], w1m[:, po, :], H2[:, po, :],
                                     start=(po == 0), stop=(po == PO - 1))
                if GELU_NATIVE:
                    nc.scalar.activation(A[:, m, :], ps1[:], AF.Gelu_apprx_tanh,
                                         bias=bf1[:, m:m + 1])
                else:
                    t0 = gp.tile([128, TPC], F32R, tag="g0")
                    nc.vector.tensor_scalar_add(t0[:], ps1[:], bf1[:, m:m + 1])
                    sq = gp.tile([128, TPC], F32R, tag="g1")
                    nc.vector.tensor_mul(sq[:], t0[:], t0[:])
                    nc.vector.tensor_scalar(out=sq[:], in0=sq[:],
                                            scalar1=0.044715, scalar2=1.0,
                                            op0=ALU.mult, op1=ALU.add)
                    nc.vector.tensor_mul(sq[:], sq[:], t0[:])
                    nc.scalar.activation(sq[:], sq[:], AF.Tanh, scale=C_GELU)
                    nc.vector.tensor_scalar(out=sq[:], in0=sq[:], scalar1=1.0,
                                            scalar2=0.5, op0=ALU.add,
                                            op1=ALU.mult)
                    nc.vector.tensor_mul(A[:, m, :], sq[:], t0[:])

            out_view = out_t.ap().rearrange("(po p) t -> p po t", p=128)
            for m in range(PO):
                w2m = w2p.tile([128, M1, 128], F32R, tag="w2")
                nc.sync.dma_start(
                    w2m[:], wf2_in.ap()[m].rearrange("(ko p) n -> p ko n", p=128))
                ps_2 = psum.tile([128, TPC], F32, tag="ps")
                for ko in range(M1):
                    nc.tensor.matmul(ps_2[:], w2m[:, ko, :], A[:, ko, :],
                                     start=(ko == 0), stop=(ko == M1 - 1))
                om = outp.tile([128, TPC], F32, tag="om")
                nc.vector.tensor_scalar_add(om[:], ps_2[:], bf2[:, m:m + 1])
                nc.vector.tensor_add(om[:], om[:], X2[:, m, :].bitcast(F32))
                nc.sync.dma_start(out_view[:, m, :], om[:])

    nc.compile()
    return nc


def _get_nc():
    key = ("nc", GELU_NATIVE)
    if key not in _CACHE:
        _CACHE[key] = _build()
    return _CACHE[key]


def _make_in_maps(inputs):
    x = np.asarray(inputs["x"], np.float32).reshape(B * T, D)
    W_qkv = np.asarray(inputs["W_qkv"], np.float32)
    b_qkv = np.asarray(inputs["b_qkv"], np.float32)
    W_o = np.asarray(inputs["W_o"], np.float32)
    b_o = np.asarray(inputs["b_o"], np.float32)
    W_ff1 = np.asarray(inputs["W_ff1"], np.float32)
    b_ff1 = np.asarray(inputs["b_ff1"], np.float32)
    W_ff2 = np.asarray(inputs["W_ff2"], np.float32)
    b_ff2 = np.asarray(inputs["b_ff2"], np.float32)

    def pcol(v):  # [D'] -> [128, D'/128] feature-major per-partition layout
        return np.ascontiguousarray(v.reshape(-1, 128).T)

    common = {
        "ln1w": pcol(np.asarray(inputs["ln1_w"], np.float32)),
        "ln1b": pcol(np.asarray(inputs["ln1_b"], np.float32)),
        "ln2w": pcol(np.asarray(inputs["ln2_w"], np.float32)),
        "ln2b": pcol(np.asarray(inputs["ln2_b"], np.float32)),
        "wo": np.ascontiguousarray(W_o.reshape(D, PO, 128).transpose(1, 0, 2)),
        "bo": pcol(b_o),
        "wf1": np.ascontiguousarray(W_ff1.reshape(D, M1, 128).transpose(1, 0, 2)),
        "bf1": pcol(b_ff1),
        "wf2": np.ascontiguousarray(W_ff2.reshape(DFF, PO, 128).transpose(1, 0, 2)),
        "bf2": pcol(b_ff2),
    }
    in_maps = []
    for r in range(NC_N):
        hc = 128 * r          # first column of this core's Q/K/V head block
        m = dict(common)
        m["xt"] = np.ascontiguousarray(x[TPC * r:TPC * (r + 1)].T)
        m["wqk"] = np.ascontiguousarray(np.concatenate(
            [W_qkv[:, hc:hc + 128], W_qkv[:, D + hc:D + hc + 128]], axis=1))
        m["bqk"] = np.ascontiguousarray(np.stack(
            [b_qkv[hc:hc + 128], b_qkv[D + hc:D + hc + 128]], axis=1))
        m["wv"] = np.ascontiguousarray(W_qkv[:, 2 * D + hc:2 * D + hc + 128])
        m["bv"] = np.ascontiguousarray(b_qkv[2 * D + hc:2 * D + hc + 128]
                                       .reshape(128, 1))
        in_maps.append(m)
    return in_maps


def _run(inputs, trace=False, trace_cores=None):
    nc = _get_nc()
    in_maps = _make_in_maps(inputs)
    res = bass_utils.run_bass_kernel_spmd(
        nc, in_maps, core_ids=list(range(NC_N)), trace=trace,
        trace_cores=trace_cores)
    outs = [res.results[r]["outt"] for r in range(NC_N)]
    full = np.concatenate([o.T for o in outs], axis=0)
    return full.reshape(B, T, D).astype(np.float32), res


def kernel(**inputs):
    out, _ = _run(inputs, trace=False)
    return out

